# revision 9
# baseline (speedup 1.0000x reference)
"""Multi-head self-attention block (B=4, N=4096, C=384, H=6, D=64) on 8 TRN2
NeuronCores.

Sharding: core c handles batch b = c // 2 and query rows
[(c % 2) * 2048, (c % 2 + 1) * 2048).  Fully data-parallel SPMD; no
collectives.

v4 over v2 (459us baseline):
  - AV restructured: P is the STATIONARY operand ([128 keys, 128 queries]
    slices) and V is the MOVING operand ([128 keys, 64+1]); the PE streams
    65 rows per matmul instead of 512, cutting AV from ~164us to ~94us
    (LDWEIGHTS of a fresh 128x128 fp16 stationary fully pipelines under a
    65-row stream: measured ~30 ns/matmul back-to-back).  The extra moving
    column of ones accumulates Z per query partition.
  - AV output is [128 queries, 65]; normalization is a per-partition
    multiply by 1/Z (reciprocal on [128, 4] directly from PSUM), then a PE
    transpose via identity puts [dims, queries] back into at_sb for the
    (unchanged) head-pair output projection.  The old DRAM-bounce Z path,
    GpSimd normalize, and odd-head SBUF DMAs are gone.
  - exp split ~half/half between ScalarE exact Exp and DVE Schraudolph
    (both run ~1.1-1.2us per [128,1024] tile; the fp32 PSUM read port is
    the limiter).  QKV projection-copy engines ALTERNATE opposite to each
    key tile's exp engine so casts never queue behind a busy exp engine
    (in v3 stalled casts blocked PSUM slot reuse and the in-order PE).
  - AV accumulators: 4 query-subtile groups share one PSUM bank per head;
    only the first (qs==0, kt==0) matmul uses start=True (bank zero-region
    wipe); later groups rely on the pending-zero region semantics.
"""

import numpy as np

import concourse.bass as bass
import concourse.tile as tile
import concourse.mybir as mybir
from concourse import bacc
from concourse.bass_utils import run_bass_kernel_spmd
from concourse.masks import make_identity

# Problem dims (hardcoded per contract)
B, N, C = 4, 4096, 384
H, D = 6, 64
SCALE = D**-0.5
NCORES = 8
NQ = N // 2  # queries per core
QB = 512  # query block
NQB = NQ // QB  # 4
QS = 128  # query subtile (AV stationary width)
NQS = QB // QS  # 4
KT = 128  # key tile
NKT = N // KT  # 32
F32 = mybir.dt.float32
F16 = mybir.dt.float16
I16 = mybir.dt.int16
F16_NP = np.float16

# Schraudolph fast-exp constants for fp16 target: exp(SCALE*s) ~=
# bitcast_f16(int16(A*s + B)); A = 2^10*log2(e)*SCALE, B = 15*2^10 - 59.
EXP_A = (2.0**10) * 1.4426950408889634 * SCALE
EXP_B = 15.0 * (2.0**10) - 59.0
# every key tile: exp split by head across BOTH engines (exact on ScalarE,
# Schraudolph on DVE), head assignment alternating with kt parity


def _build():
    nc = bacc.Bacc(None, target_bir_lowering=False)

    xt = nc.dram_tensor("xt", [3, 128, N], F16, kind="ExternalInput")
    xq = nc.dram_tensor("xq", [3, 128, NQ], F16, kind="ExternalInput")
    wqkv = nc.dram_tensor("wqkv", [3, 128, 3 * C], F16, kind="ExternalInput")
    wproj = nc.dram_tensor("wproj", [3, 128, C], F16, kind="ExternalInput")
    bproj = nc.dram_tensor("bproj", [128, 3], F32, kind="ExternalInput")
    out_t = nc.dram_tensor("out_t", [3, 128, NQ], F32, kind="ExternalOutput")

    with tile.TileContext(nc) as tc:
        with (
            tc.tile_pool(name="persist", bufs=1) as persist,
            tc.tile_pool(name="ppool", bufs=5) as ppool,
            tc.tile_pool(name="zpool", bufs=8) as zpool,
            tc.tile_pool(name="apool", bufs=12) as apool,
            tc.tile_pool(name="ypool", bufs=3) as ypool,
            tc.tile_pool(name="spool", bufs=2, space="PSUM") as spool,
            tc.tile_pool(name="opool", bufs=1, space="PSUM") as opool,
            tc.tile_pool(name="wpool", bufs=2, space="PSUM") as wpool,
        ):
            # ---- persistent SBUF tensors ----
            xt_sb = [persist.tile([128, N], F16, tag=f"xt{c}", name=f"xt_sb{c}") for c in range(3)]
            xq_sb = [persist.tile([128, NQ], F16, tag=f"xq{c}", name=f"xq_sb{c}") for c in range(3)]
            wq_sb = persist.tile([128, 3, 3 * C], F16, tag="wq")
            wp_sb = persist.tile([128, 3, C], F16, tag="wp")
            bp_sb = persist.tile([128, 3], F32, tag="bp")
            kt_sb = [persist.tile([128, N], F16, tag=f"kt{j}", name=f"kt_sb{j}") for j in range(3)]
            qt_sb = [persist.tile([128, NQ], F16, tag=f"qt{j}", name=f"qt_sb{j}") for j in range(3)]
            vp_sb = [
                persist.tile([128, H, D + 1], F16, tag=f"vp{k}", name=f"vp_sb{k}")
                for k in range(NKT)
            ]
            at_sb = [persist.tile([128, NQ], F16, tag=f"at{j}", name=f"at_sb{j}") for j in range(3)]
            ident_f32 = persist.tile([128, 128], F32, tag="ident")
            make_identity(nc, ident_f32)

            # ---- input DMAs, ordered so the first QK matmuls start ASAP ----
            for c in range(3):
                nc.sync.dma_start(out=wq_sb[:, c, 0 : 2 * C], in_=wqkv[c, :, 0 : 2 * C])
            for c in range(3):
                nc.scalar.dma_start(out=xt_sb[c][:, 0:1024], in_=xt[c, :, 0:1024])
            for c in range(3):
                nc.gpsimd.dma_start(out=xq_sb[c][:, 0:512], in_=xq[c, :, 0:512])
            for c in range(3):
                nc.sync.dma_start(out=wq_sb[:, c, 2 * C : 3 * C], in_=wqkv[c, :, 2 * C : 3 * C])
            for t in range(1, 4):
                for c in range(3):
                    nc.gpsimd.dma_start(
                        out=xt_sb[c][:, t * 1024 : (t + 1) * 1024],
                        in_=xt[c, :, t * 1024 : (t + 1) * 1024],
                    )
            for c in range(3):
                nc.gpsimd.dma_start(out=xq_sb[c][:, 512:NQ], in_=xq[c, :, 512:NQ])
            for hp in range(3):
                nc.gpsimd.dma_start(out=wp_sb[:, hp, :], in_=wproj[hp, :, :])
            nc.gpsimd.dma_start(out=bp_sb, in_=bproj[:, :])

            # ---- QKV projection units (cast engine chosen per call site) ----
            def v_unit(k, eng):
                vps = wpool.tile([128, C], F32, tag="work", name=f"vps{k}")
                for c in range(3):
                    nc.tensor.matmul(
                        vps,
                        xt_sb[c][:, k * KT : (k + 1) * KT],
                        wq_sb[:, c, 2 * C : 3 * C],
                        start=(c == 0),
                        stop=(c == 2),
                    )
                dst = vp_sb[k]
                if eng == "v":
                    nc.vector.tensor_copy(
                        out=dst[:, :, 0:D], in_=vps.rearrange("p (h d) -> p h d", d=D)
                    )
                else:
                    nc.scalar.copy(
                        out=dst[:, :, 0:D], in_=vps.rearrange("p (h d) -> p h d", d=D)
                    )
                nc.gpsimd.memset(dst[:, :, D : D + 1], 1.0)

            def kq_unit(kind, j, t, eng):
                kq = wpool.tile([128, 512], F32, tag="work", name=f"kq{kind}{j}_{t}")
                coff = C + j * 128 if kind == "k" else j * 128
                src = xt_sb if kind == "k" else xq_sb
                dstt = kt_sb[j] if kind == "k" else qt_sb[j]
                for c in range(3):
                    nc.tensor.matmul(
                        kq,
                        wq_sb[:, c, coff : coff + 128],
                        src[c][:, t * 512 : (t + 1) * 512],
                        start=(c == 0),
                        stop=(c == 2),
                    )
                if eng == "v":
                    nc.vector.tensor_copy(out=dstt[:, t * 512 : (t + 1) * 512], in_=kq)
                else:
                    nc.scalar.copy(out=dstt[:, t * 512 : (t + 1) * 512], in_=kq)

            # upfront: only head pair 0's first K/Q tiles; ALL other QKV
            # work drains into the attention stream (PE is in-order).
            kq_unit("k", 0, 0, "v")
            kq_unit("q", 0, 0, "s")

            units = []
            ks = [("k", 0, t) for t in range(1, N // 512)]
            for k in range(NKT):
                units.append(("v", k, 0))
                if k % 2 == 1 and ks:
                    units.append(ks.pop(0))
            units.extend(ks)
            for j in (1, 2):
                units.append(("k", j, 0))
                units.append(("q", j, 0))
                for t in range(1, N // 512):
                    units.append(("k", j, t))
                for t in range(1, NQ // 512):
                    units.append(("q", j, t))
            for t in range(1, NQ // 512):
                units.append(("q", 0, t))
            units.reverse()  # pop() from the end

            def emit_unit(u, eng):
                if u[0] == "v":
                    v_unit(u[1], eng)
                else:
                    kq_unit(u[0], u[1], u[2], eng)

            # ---- deferred output projection (head-pair contraction K=128) ----
            def make_proj(qb):
                q0 = qb * QB

                def emit(co):
                    y = wpool.tile([128, QB], F32, tag="work", name=f"y{qb}_{co}")
                    for hp in range(3):
                        nc.tensor.matmul(
                            y,
                            wp_sb[:, hp, co * 128 : (co + 1) * 128],
                            at_sb[hp][:, q0 : q0 + QB],
                            start=(hp == 0),
                            stop=(hp == 2),
                        )
                    ysb = ypool.tile([128, QB], F32, tag="y", name=f"ysb{qb}_{co}")
                    # fused bias add during the PSUM->SBUF copy
                    nc.vector.tensor_scalar(
                        ysb, y, bp_sb[:, co : co + 1], None, mybir.AluOpType.add
                    )
                    nc.sync.dma_start(out=out_t[co, :, q0 : q0 + QB], in_=ysb)

                return emit

            pending_proj = None
            tqueue = []  # deferred (transpose + at-copy) work items

            def drain_transpose():
                t_att, t_hp, t_q0, t_qs = tqueue.pop(0)
                tp = wpool.tile(
                    [128, 128], F32, tag="work", name=f"tp{t_hp}_{t_q0}_{t_qs}"
                )
                nc.tensor.transpose(tp, t_att, ident_f32)
                dst = at_sb[t_hp][:, t_q0 + t_qs * QS : t_q0 + (t_qs + 1) * QS]
                nc.scalar.copy(out=dst, in_=tp)

            # ---- attention: (query-512-block, head-pair) x 32 key tiles ----
            for qb in range(NQB):
                q0 = qb * QB
                for hp in range(3):
                    hA, hB = 2 * hp, 2 * hp + 1
                    # one PSUM bank per head: [128 queries, 4 qsubtiles, 65]
                    oaccA = opool.tile([128, NQS, D + 1], F32, tag="oaccA", name=f"oA{qb}_{hp}")
                    oaccB = opool.tile([128, NQS, D + 1], F32, tag="oaccB", name=f"oB{qb}_{hp}")
                    pendings = []

                    def emit_av(pk, pp):
                        for oacc, h, off in ((oaccA, hA, 0), (oaccB, hB, QB)):
                            for qs in range(NQS):
                                nc.tensor.matmul(
                                    oacc[:, qs, :],
                                    pp[:, off + qs * QS : off + (qs + 1) * QS],
                                    vp_sb[pk][:, h, :],
                                    start=(pk == 0 and qs == 0),
                                    stop=(pk == NKT - 1 and qs == NQS - 1),
                                    skip_group_check=True,
                                )

                    for k in range(NKT):
                        cast_eng = "v" if k % 2 == 0 else "s"
                        s = spool.tile([128, 2 * QB], F32, tag="s")
                        nc.tensor.matmul(
                            s[:, 0:QB],
                            kt_sb[hp][0:D, k * KT : (k + 1) * KT],
                            qt_sb[hp][0:D, q0 : q0 + QB],
                            start=True,
                            stop=True,
                        )
                        nc.tensor.matmul(
                            s[:, QB : 2 * QB],
                            kt_sb[hp][D : 2 * D, k * KT : (k + 1) * KT],
                            qt_sb[hp][D : 2 * D, q0 : q0 + QB],
                            start=True,
                            stop=True,
                        )
                        p = ppool.tile([128, 2 * QB], F16, tag="p")
                        # split the tile's exp across both engines by head;
                        # swap which head is exact every kt to decorrelate
                        ex_sl = slice(0, QB) if k % 2 == 0 else slice(QB, 2 * QB)
                        fa_sl = slice(QB, 2 * QB) if k % 2 == 0 else slice(0, QB)
                        nc.scalar.activation(
                            p[:, ex_sl],
                            s[:, ex_sl],
                            mybir.ActivationFunctionType.Exp,
                            scale=SCALE,
                        )
                        nc.vector.tensor_scalar(
                            p.bitcast(I16)[:, fa_sl],
                            s[:, fa_sl],
                            EXP_A,
                            EXP_B,
                            mybir.AluOpType.mult,
                            mybir.AluOpType.add,
                        )
                        if len(pendings) >= 2:
                            emit_av(*pendings.pop(0))
                        pendings.append((k, p))
                        if tqueue and k in (1, 3, 5, 7):
                            drain_transpose()
                        # drain deferred QKV work into the stream (PE slack)
                        if units and len(units) > 32:
                            emit_unit(units.pop(), cast_eng)
                            emit_unit(units.pop(), "v" if cast_eng == "s" else "s")
                        elif units and k % 2 == 1:
                            emit_unit(units.pop(), cast_eng)
                        if pending_proj is not None and hp == 0 and k in (8, 14, 20):
                            pending_proj({8: 0, 14: 1, 20: 2}[k])
                            if k == 20:
                                pending_proj = None
                    for pk, pp in pendings:
                        emit_av(pk, pp)
                    # normalize (1/Z per query partition) + PE transpose back
                    # to [dims, queries] for the projection
                    rzA = zpool.tile([128, NQS], F32, tag="rzA", name=f"rzA{qb}_{hp}")
                    rzB = zpool.tile([128, NQS], F32, tag="rzB", name=f"rzB{qb}_{hp}")
                    nc.vector.reciprocal(out=rzA, in_=oaccA[:, :, D])
                    nc.vector.reciprocal(out=rzB, in_=oaccB[:, :, D])
                    for qs in range(NQS):
                        att = apool.tile([128, 128], F32, tag="att", name=f"att{qb}_{hp}_{qs}")
                        nc.vector.tensor_scalar(
                            att[:, 0:D],
                            oaccA[:, qs, 0:D],
                            rzA[:, qs : qs + 1],
                            None,
                            mybir.AluOpType.mult,
                        )
                        nc.vector.tensor_scalar(
                            att[:, D : 2 * D],
                            oaccB[:, qs, 0:D],
                            rzB[:, qs : qs + 1],
                            None,
                            mybir.AluOpType.mult,
                        )
                        tqueue.append((att, hp, q0, qs))
                if qb < NQB - 1:
                    pending_proj = make_proj(qb)
            while tqueue:
                drain_transpose()
            final_proj = make_proj(NQB - 1)
            for co in range(3):
                final_proj(co)

    nc.compile()
    return nc


_NC_CACHE = {}


def _get_nc():
    if "nc" not in _NC_CACHE:
        _NC_CACHE["nc"] = _build()
    return _NC_CACHE["nc"]


def _prep_core_inputs(x, w_qkv, w_proj, b_proj):
    """Host-side sharding: returns in_maps for the 8 cores."""
    wqkv_p = np.ascontiguousarray(w_qkv.reshape(3, 128, 3 * C)).astype(F16_NP)
    wproj_p = np.ascontiguousarray(w_proj.reshape(3, 128, C)).astype(F16_NP)
    bproj_p = np.ascontiguousarray(b_proj.reshape(3, 128).T).astype(np.float32)
    in_maps = []
    for core in range(NCORES):
        b, qh = core // 2, core % 2
        xt_b = np.ascontiguousarray(x[b].T).astype(F16_NP)  # [C, N]
        xq_b = np.ascontiguousarray(x[b, qh * NQ : (qh + 1) * NQ].T).astype(F16_NP)
        in_maps.append(
            {
                "xt": xt_b.reshape(3, 128, N),
                "xq": xq_b.reshape(3, 128, NQ),
                "wqkv": wqkv_p,
                "wproj": wproj_p,
                "bproj": bproj_p,
            }
        )
    return in_maps


def run(inputs, **kw):
    """Run the kernel; returns (full_output, BassKernelResults)."""
    x = np.asarray(inputs["x"], dtype=np.float32)
    w_qkv = np.asarray(inputs["w_qkv"], dtype=np.float32)
    w_proj = np.asarray(inputs["w_proj"], dtype=np.float32)
    b_proj = np.asarray(inputs["b_proj"], dtype=np.float32)

    nc = _get_nc()
    in_maps = _prep_core_inputs(x, w_qkv, w_proj, b_proj)
    res = run_bass_kernel_spmd(nc, in_maps, core_ids=list(range(NCORES)), **kw)

    out = np.empty((B, N, C), dtype=np.float32)
    for core in range(NCORES):
        b, qh = core // 2, core % 2
        yt = res.results[core]["out_t"].reshape(C, NQ)  # [3*128, NQ]
        out[b, qh * NQ : (qh + 1) * NQ, :] = yt.T
    return out, res


def kernel(**inputs) -> np.ndarray:
    out, _ = run(inputs)
    return out


# revision 10
# speedup vs baseline: 1.2528x; 1.2528x over previous
"""Multi-head self-attention block (B=4, N=4096, C=384, H=6, D=64) on 8 TRN2
NeuronCores.

Sharding: core c handles batch b = c // 2 and query rows
[(c % 2) * 2048, (c % 2 + 1) * 2048).  Fully data-parallel SPMD; no
collectives.

v4 over v2 (459us baseline):
  - AV restructured: P is the STATIONARY operand ([128 keys, 128 queries]
    slices) and V is the MOVING operand ([128 keys, 64+1]); the PE streams
    65 rows per matmul instead of 512, cutting AV from ~164us to ~94us
    (LDWEIGHTS of a fresh 128x128 fp16 stationary fully pipelines under a
    65-row stream: measured ~30 ns/matmul back-to-back).  The extra moving
    column of ones accumulates Z per query partition.
  - AV output is [128 queries, 65]; normalization is a per-partition
    multiply by 1/Z (reciprocal on [128, 4] directly from PSUM), then a PE
    transpose via identity puts [dims, queries] back into at_sb for the
    (unchanged) head-pair output projection.  The old DRAM-bounce Z path,
    GpSimd normalize, and odd-head SBUF DMAs are gone.
  - exp split ~half/half between ScalarE exact Exp and DVE Schraudolph
    (both run ~1.1-1.2us per [128,1024] tile; the fp32 PSUM read port is
    the limiter).  QKV projection-copy engines ALTERNATE opposite to each
    key tile's exp engine so casts never queue behind a busy exp engine
    (in v3 stalled casts blocked PSUM slot reuse and the in-order PE).
  - AV accumulators: 4 query-subtile groups share one PSUM bank per head;
    only the first (qs==0, kt==0) matmul uses start=True (bank zero-region
    wipe); later groups rely on the pending-zero region semantics.
"""

import numpy as np

import concourse.bass as bass
import concourse.tile as tile
import concourse.mybir as mybir
from concourse import bacc
from concourse.bass_utils import run_bass_kernel_spmd
from concourse.masks import make_identity

# Problem dims (hardcoded per contract)
B, N, C = 4, 4096, 384
H, D = 6, 64
SCALE = D**-0.5
NCORES = 8
NQ = N // 2  # queries per core
QB = 512  # query block
NQB = NQ // QB  # 4
QS = 128  # query subtile (AV stationary width)
NQS = QB // QS  # 4
KT = 128  # key tile
NKT = N // KT  # 32
F32 = mybir.dt.float32
F16 = mybir.dt.float16
I16 = mybir.dt.int16
F16_NP = np.float16

# Schraudolph fast-exp constants for fp16 target: exp(SCALE*s) ~=
# bitcast_f16(int16(A*s + B)); A = 2^10*log2(e)*SCALE, B = 15*2^10 - 59.
EXP_A = (2.0**10) * 1.4426950408889634 * SCALE
EXP_B = 15.0 * (2.0**10) - 59.0
# key tiles strictly alternate exp engines: even kt -> ScalarE exact Exp,
# odd kt -> DVE Schraudolph; consecutive tiles run concurrently on the two
# engines (different PSUM banks, no read-port contention)


def _build():
    nc = bacc.Bacc(None, target_bir_lowering=False)

    xt = nc.dram_tensor("xt", [3, 128, N], F16, kind="ExternalInput")
    xq = nc.dram_tensor("xq", [3, 128, NQ], F16, kind="ExternalInput")
    wqkv = nc.dram_tensor("wqkv", [3, 128, 3 * C], F16, kind="ExternalInput")
    wproj = nc.dram_tensor("wproj", [3, 128, C], F16, kind="ExternalInput")
    bproj = nc.dram_tensor("bproj", [128, 3], F32, kind="ExternalInput")
    out_t = nc.dram_tensor("out_t", [3, 128, NQ], F32, kind="ExternalOutput")

    with tile.TileContext(nc) as tc:
        with (
            tc.tile_pool(name="persist", bufs=1) as persist,
            tc.tile_pool(name="ppool", bufs=5) as ppool,
            tc.tile_pool(name="zpool", bufs=8) as zpool,
            tc.tile_pool(name="apool", bufs=12) as apool,
            tc.tile_pool(name="ypool", bufs=3) as ypool,
            tc.tile_pool(name="spool", bufs=2, space="PSUM") as spool,
            tc.tile_pool(name="opool", bufs=1, space="PSUM") as opool,
            tc.tile_pool(name="wpool", bufs=2, space="PSUM") as wpool,
        ):
            # ---- persistent SBUF tensors ----
            xt_sb = [persist.tile([128, N], F16, tag=f"xt{c}", name=f"xt_sb{c}") for c in range(3)]
            xq_sb = [persist.tile([128, NQ], F16, tag=f"xq{c}", name=f"xq_sb{c}") for c in range(3)]
            wq_sb = persist.tile([128, 3, 3 * C], F16, tag="wq")
            wp_sb = persist.tile([128, 3, C], F16, tag="wp")
            bp_sb = persist.tile([128, 3], F32, tag="bp")
            kt_sb = [persist.tile([128, N], F16, tag=f"kt{j}", name=f"kt_sb{j}") for j in range(3)]
            qt_sb = [persist.tile([128, NQ], F16, tag=f"qt{j}", name=f"qt_sb{j}") for j in range(3)]
            vp_sb = [
                persist.tile([128, H, D + 1], F16, tag=f"vp{k}", name=f"vp_sb{k}")
                for k in range(NKT)
            ]
            at_sb = [persist.tile([128, NQ], F16, tag=f"at{j}", name=f"at_sb{j}") for j in range(3)]
            ident = persist.tile([128, 128], F16, tag="ident")
            make_identity(nc, ident)

            # ---- input DMAs, ordered so the first QK matmuls start ASAP ----
            for c in range(3):
                nc.sync.dma_start(out=wq_sb[:, c, 0 : 2 * C], in_=wqkv[c, :, 0 : 2 * C])
            for c in range(3):
                nc.scalar.dma_start(out=xt_sb[c][:, 0:1024], in_=xt[c, :, 0:1024])
            for c in range(3):
                nc.gpsimd.dma_start(out=xq_sb[c][:, 0:512], in_=xq[c, :, 0:512])
            for c in range(3):
                nc.sync.dma_start(out=wq_sb[:, c, 2 * C : 3 * C], in_=wqkv[c, :, 2 * C : 3 * C])
            for t in range(1, 4):
                for c in range(3):
                    nc.gpsimd.dma_start(
                        out=xt_sb[c][:, t * 1024 : (t + 1) * 1024],
                        in_=xt[c, :, t * 1024 : (t + 1) * 1024],
                    )
            for c in range(3):
                nc.gpsimd.dma_start(out=xq_sb[c][:, 512:NQ], in_=xq[c, :, 512:NQ])
            for hp in range(3):
                nc.gpsimd.dma_start(out=wp_sb[:, hp, :], in_=wproj[hp, :, :])
            nc.gpsimd.dma_start(out=bp_sb, in_=bproj[:, :])

            # ---- QKV projection units (cast engine chosen per call site) ----
            def v_unit(k, eng):
                vps = wpool.tile([128, C], F32, tag="work", name=f"vps{k}")
                for c in range(3):
                    nc.tensor.matmul(
                        vps,
                        xt_sb[c][:, k * KT : (k + 1) * KT],
                        wq_sb[:, c, 2 * C : 3 * C],
                        start=(c == 0),
                        stop=(c == 2),
                    )
                dst = vp_sb[k]
                if eng == "v":
                    nc.vector.tensor_copy(
                        out=dst[:, :, 0:D], in_=vps.rearrange("p (h d) -> p h d", d=D)
                    )
                else:
                    nc.scalar.copy(
                        out=dst[:, :, 0:D], in_=vps.rearrange("p (h d) -> p h d", d=D)
                    )
                nc.gpsimd.memset(dst[:, :, D : D + 1], 1.0)

            def kq_unit(kind, j, t, eng):
                kq = wpool.tile([128, 512], F32, tag="work", name=f"kq{kind}{j}_{t}")
                coff = C + j * 128 if kind == "k" else j * 128
                src = xt_sb if kind == "k" else xq_sb
                dstt = kt_sb[j] if kind == "k" else qt_sb[j]
                for c in range(3):
                    nc.tensor.matmul(
                        kq,
                        wq_sb[:, c, coff : coff + 128],
                        src[c][:, t * 512 : (t + 1) * 512],
                        start=(c == 0),
                        stop=(c == 2),
                    )
                if eng == "v":
                    nc.vector.tensor_copy(out=dstt[:, t * 512 : (t + 1) * 512], in_=kq)
                else:
                    nc.scalar.copy(out=dstt[:, t * 512 : (t + 1) * 512], in_=kq)

            # upfront: only head pair 0's first K/Q tiles; ALL other QKV
            # work drains into the attention stream (PE is in-order).
            kq_unit("k", 0, 0, "v")
            kq_unit("q", 0, 0, "s")

            units = []
            ks = [("k", 0, t) for t in range(1, N // 512)]
            for k in range(NKT):
                units.append(("v", k, 0))
                if k % 2 == 1 and ks:
                    units.append(ks.pop(0))
            units.extend(ks)
            for j in (1, 2):
                units.append(("k", j, 0))
                units.append(("q", j, 0))
                for t in range(1, N // 512):
                    units.append(("k", j, t))
                for t in range(1, NQ // 512):
                    units.append(("q", j, t))
            for t in range(1, NQ // 512):
                units.append(("q", 0, t))
            units.reverse()  # pop() from the end

            def emit_unit(u, eng):
                if u[0] == "v":
                    v_unit(u[1], eng)
                else:
                    kq_unit(u[0], u[1], u[2], eng)

            # ---- deferred output projection (head-pair contraction K=128) ----
            def make_proj(qb):
                q0 = qb * QB

                def emit(co):
                    y = wpool.tile([128, QB], F32, tag="work", name=f"y{qb}_{co}")
                    for hp in range(3):
                        nc.tensor.matmul(
                            y,
                            wp_sb[:, hp, co * 128 : (co + 1) * 128],
                            at_sb[hp][:, q0 : q0 + QB],
                            start=(hp == 0),
                            stop=(hp == 2),
                        )
                    ysb = ypool.tile([128, QB], F32, tag="y", name=f"ysb{qb}_{co}")
                    # fused bias add during the PSUM->SBUF copy
                    nc.vector.tensor_scalar(
                        ysb, y, bp_sb[:, co : co + 1], None, mybir.AluOpType.add
                    )
                    nc.sync.dma_start(out=out_t[co, :, q0 : q0 + QB], in_=ysb)

                return emit

            pending_proj = None
            tqueue = []  # deferred (transpose + at-copy) work items

            def drain_transpose():
                t_att, t_hp, t_q0, t_qs = tqueue.pop(0)
                tp = wpool.tile(
                    [128, 128], F16, tag="work", name=f"tp{t_hp}_{t_q0}_{t_qs}"
                )
                nc.tensor.transpose(tp, t_att, ident)
                dst = at_sb[t_hp][:, t_q0 + t_qs * QS : t_q0 + (t_qs + 1) * QS]
                nc.scalar.copy(out=dst, in_=tp)

            # ---- attention: (query-512-block, head-pair) x 32 key tiles ----
            for qb in range(NQB):
                q0 = qb * QB
                for hp in range(3):
                    hA, hB = 2 * hp, 2 * hp + 1
                    # one PSUM bank per head: [128 queries, 4 qsubtiles, 65]
                    oaccA = opool.tile([128, NQS, D + 1], F32, tag="oaccA", name=f"oA{qb}_{hp}")
                    oaccB = opool.tile([128, NQS, D + 1], F32, tag="oaccB", name=f"oB{qb}_{hp}")
                    pendings = []

                    def emit_av(pk, pp):
                        for oacc, h, off in ((oaccA, hA, 0), (oaccB, hB, QB)):
                            for qs in range(NQS):
                                nc.tensor.matmul(
                                    oacc[:, qs, :],
                                    pp[:, off + qs * QS : off + (qs + 1) * QS],
                                    vp_sb[pk][:, h, :],
                                    start=(pk == 0 and qs == 0),
                                    stop=(pk == NKT - 1 and qs == NQS - 1),
                                    skip_group_check=True,
                                )

                    for k in range(NKT):
                        cast_eng = "v" if k % 2 == 0 else "s"
                        s = spool.tile([128, 2 * QB], F32, tag="s")
                        nc.tensor.matmul(
                            s[:, 0:QB],
                            kt_sb[hp][0:D, k * KT : (k + 1) * KT],
                            qt_sb[hp][0:D, q0 : q0 + QB],
                            start=True,
                            stop=True,
                        )
                        nc.tensor.matmul(
                            s[:, QB : 2 * QB],
                            kt_sb[hp][D : 2 * D, k * KT : (k + 1) * KT],
                            qt_sb[hp][D : 2 * D, q0 : q0 + QB],
                            start=True,
                            stop=True,
                        )
                        p = ppool.tile([128, 2 * QB], F16, tag="p")
                        if k % 2 == 0:
                            nc.scalar.activation(
                                p, s, mybir.ActivationFunctionType.Exp, scale=SCALE
                            )
                        else:
                            nc.vector.tensor_scalar(
                                p.bitcast(I16),
                                s,
                                EXP_A,
                                EXP_B,
                                mybir.AluOpType.mult,
                                mybir.AluOpType.add,
                            )
                        if len(pendings) >= 2:
                            emit_av(*pendings.pop(0))
                        pendings.append((k, p))
                        if tqueue and k in (1, 3, 5, 7):
                            drain_transpose()
                        # drain deferred QKV work into the stream (PE slack)
                        if units and len(units) > 32:
                            emit_unit(units.pop(), cast_eng)
                            emit_unit(units.pop(), "v" if cast_eng == "s" else "s")
                        elif units and k % 2 == 1:
                            emit_unit(units.pop(), cast_eng)
                        if pending_proj is not None and hp == 0 and k in (8, 14, 20):
                            pending_proj({8: 0, 14: 1, 20: 2}[k])
                            if k == 20:
                                pending_proj = None
                    for pk, pp in pendings:
                        emit_av(pk, pp)
                    # normalize (1/Z per query partition) + PE transpose back
                    # to [dims, queries] for the projection
                    rzA = zpool.tile([128, NQS], F32, tag="rzA", name=f"rzA{qb}_{hp}")
                    rzB = zpool.tile([128, NQS], F32, tag="rzB", name=f"rzB{qb}_{hp}")
                    nc.vector.reciprocal(out=rzA, in_=oaccA[:, :, D])
                    nc.vector.reciprocal(out=rzB, in_=oaccB[:, :, D])
                    for qs in range(NQS):
                        att = apool.tile([128, 128], F16, tag="att", name=f"att{qb}_{hp}_{qs}")
                        nc.vector.tensor_scalar(
                            att[:, 0:D],
                            oaccA[:, qs, 0:D],
                            rzA[:, qs : qs + 1],
                            None,
                            mybir.AluOpType.mult,
                        )
                        nc.vector.tensor_scalar(
                            att[:, D : 2 * D],
                            oaccB[:, qs, 0:D],
                            rzB[:, qs : qs + 1],
                            None,
                            mybir.AluOpType.mult,
                        )
                        tqueue.append((att, hp, q0, qs))
                if qb < NQB - 1:
                    pending_proj = make_proj(qb)
            while tqueue:
                drain_transpose()
            final_proj = make_proj(NQB - 1)
            for co in range(3):
                final_proj(co)

    nc.compile()
    return nc


_NC_CACHE = {}


def _get_nc():
    if "nc" not in _NC_CACHE:
        _NC_CACHE["nc"] = _build()
    return _NC_CACHE["nc"]


def _prep_core_inputs(x, w_qkv, w_proj, b_proj):
    """Host-side sharding: returns in_maps for the 8 cores."""
    wqkv_p = np.ascontiguousarray(w_qkv.reshape(3, 128, 3 * C)).astype(F16_NP)
    wproj_p = np.ascontiguousarray(w_proj.reshape(3, 128, C)).astype(F16_NP)
    bproj_p = np.ascontiguousarray(b_proj.reshape(3, 128).T).astype(np.float32)
    in_maps = []
    for core in range(NCORES):
        b, qh = core // 2, core % 2
        xt_b = np.ascontiguousarray(x[b].T).astype(F16_NP)  # [C, N]
        xq_b = np.ascontiguousarray(x[b, qh * NQ : (qh + 1) * NQ].T).astype(F16_NP)
        in_maps.append(
            {
                "xt": xt_b.reshape(3, 128, N),
                "xq": xq_b.reshape(3, 128, NQ),
                "wqkv": wqkv_p,
                "wproj": wproj_p,
                "bproj": bproj_p,
            }
        )
    return in_maps


def run(inputs, **kw):
    """Run the kernel; returns (full_output, BassKernelResults)."""
    x = np.asarray(inputs["x"], dtype=np.float32)
    w_qkv = np.asarray(inputs["w_qkv"], dtype=np.float32)
    w_proj = np.asarray(inputs["w_proj"], dtype=np.float32)
    b_proj = np.asarray(inputs["b_proj"], dtype=np.float32)

    nc = _get_nc()
    in_maps = _prep_core_inputs(x, w_qkv, w_proj, b_proj)
    res = run_bass_kernel_spmd(nc, in_maps, core_ids=list(range(NCORES)), **kw)

    out = np.empty((B, N, C), dtype=np.float32)
    for core in range(NCORES):
        b, qh = core // 2, core % 2
        yt = res.results[core]["out_t"].reshape(C, NQ)  # [3*128, NQ]
        out[b, qh * NQ : (qh + 1) * NQ, :] = yt.T
    return out, res


def kernel(**inputs) -> np.ndarray:
    out, _ = run(inputs)
    return out


# revision 11
# speedup vs baseline: 1.2861x; 1.0266x over previous
"""Multi-head self-attention block (B=4, N=4096, C=384, H=6, D=64) on 8 TRN2
NeuronCores.

Sharding: core c handles batch b = c // 2 and query rows
[(c % 2) * 2048, (c % 2 + 1) * 2048).  Fully data-parallel SPMD; no
collectives.

v4 over v2 (459us baseline):
  - AV restructured: P is the STATIONARY operand ([128 keys, 128 queries]
    slices) and V is the MOVING operand ([128 keys, 64+1]); the PE streams
    65 rows per matmul instead of 512, cutting AV from ~164us to ~94us
    (LDWEIGHTS of a fresh 128x128 fp16 stationary fully pipelines under a
    65-row stream: measured ~30 ns/matmul back-to-back).  The extra moving
    column of ones accumulates Z per query partition.
  - AV output is [128 queries, 65]; normalization is a per-partition
    multiply by 1/Z (reciprocal on [128, 4] directly from PSUM), then a PE
    transpose via identity puts [dims, queries] back into at_sb for the
    (unchanged) head-pair output projection.  The old DRAM-bounce Z path,
    GpSimd normalize, and odd-head SBUF DMAs are gone.
  - exp split ~half/half between ScalarE exact Exp and DVE Schraudolph
    (both run ~1.1-1.2us per [128,1024] tile; the fp32 PSUM read port is
    the limiter).  QKV projection-copy engines ALTERNATE opposite to each
    key tile's exp engine so casts never queue behind a busy exp engine
    (in v3 stalled casts blocked PSUM slot reuse and the in-order PE).
  - AV accumulators: 4 query-subtile groups share one PSUM bank per head;
    only the first (qs==0, kt==0) matmul uses start=True (bank zero-region
    wipe); later groups rely on the pending-zero region semantics.
"""

import numpy as np

import concourse.bass as bass
import concourse.tile as tile
import concourse.mybir as mybir
from concourse import bacc
from concourse.bass_utils import run_bass_kernel_spmd
from concourse.masks import make_identity

# Problem dims (hardcoded per contract)
B, N, C = 4, 4096, 384
H, D = 6, 64
SCALE = D**-0.5
NCORES = 8
NQ = N // 2  # queries per core
QB = 512  # query block
NQB = NQ // QB  # 4
QS = 128  # query subtile (AV stationary width)
NQS = QB // QS  # 4
KT = 128  # key tile
NKT = N // KT  # 32
F32 = mybir.dt.float32
F16 = mybir.dt.float16
I16 = mybir.dt.int16
F16_NP = np.float16

# Schraudolph fast-exp constants for fp16 target: exp(SCALE*s) ~=
# bitcast_f16(int16(A*s + B)); A = 2^10*log2(e)*SCALE, B = 15*2^10 - 59.
EXP_A = (2.0**10) * 1.4426950408889634 * SCALE
EXP_B = 15.0 * (2.0**10) - 59.0
# exp engines alternate per key tile (ScalarE exact / DVE Schraudolph) so
# consecutive tiles pipeline on both engines concurrently; 17/32 exact with
# one isolated ScalarE double (parity flip at kt 16) to shift load to the
# slightly faster engine without back-to-back runs
EXACT_TILE = tuple((k % 2 == 0) if k < 16 else (k == 16 or k % 2 == 1) for k in range(NKT))


def _build():
    nc = bacc.Bacc(None, target_bir_lowering=False)

    xt = nc.dram_tensor("xt", [3, 128, N], F16, kind="ExternalInput")
    xq = nc.dram_tensor("xq", [3, 128, NQ], F16, kind="ExternalInput")
    wqkv = nc.dram_tensor("wqkv", [3, 128, 3 * C], F16, kind="ExternalInput")
    wproj = nc.dram_tensor("wproj", [3, 128, C], F16, kind="ExternalInput")
    bproj = nc.dram_tensor("bproj", [128, 3], F32, kind="ExternalInput")
    out_t = nc.dram_tensor("out_t", [3, 128, NQ], F32, kind="ExternalOutput")

    with tile.TileContext(nc) as tc:
        with (
            tc.tile_pool(name="persist", bufs=1) as persist,
            tc.tile_pool(name="ppool", bufs=5) as ppool,
            tc.tile_pool(name="zpool", bufs=8) as zpool,
            tc.tile_pool(name="apool", bufs=12) as apool,
            tc.tile_pool(name="ypool", bufs=3) as ypool,
            tc.tile_pool(name="spool", bufs=2, space="PSUM") as spool,
            tc.tile_pool(name="opool", bufs=1, space="PSUM") as opool,
            tc.tile_pool(name="wpool", bufs=2, space="PSUM") as wpool,
        ):
            # ---- persistent SBUF tensors ----
            xt_sb = [persist.tile([128, N], F16, tag=f"xt{c}", name=f"xt_sb{c}") for c in range(3)]
            xq_sb = [persist.tile([128, NQ], F16, tag=f"xq{c}", name=f"xq_sb{c}") for c in range(3)]
            wq_sb = persist.tile([128, 3, 3 * C], F16, tag="wq")
            wp_sb = persist.tile([128, 3, C], F16, tag="wp")
            bp_sb = persist.tile([128, 3], F32, tag="bp")
            kt_sb = [persist.tile([128, N], F16, tag=f"kt{j}", name=f"kt_sb{j}") for j in range(3)]
            qt_sb = [persist.tile([128, NQ], F16, tag=f"qt{j}", name=f"qt_sb{j}") for j in range(3)]
            vp_sb = [
                persist.tile([128, H, D + 1], F16, tag=f"vp{k}", name=f"vp_sb{k}")
                for k in range(NKT)
            ]
            at_sb = [persist.tile([128, NQ], F16, tag=f"at{j}", name=f"at_sb{j}") for j in range(3)]
            ident = persist.tile([128, 128], F16, tag="ident")
            make_identity(nc, ident)

            # ---- input DMAs, ordered so the first QK matmuls start ASAP ----
            for c in range(3):
                nc.sync.dma_start(out=wq_sb[:, c, 0 : 2 * C], in_=wqkv[c, :, 0 : 2 * C])
            for c in range(3):
                nc.scalar.dma_start(out=xt_sb[c][:, 0:1024], in_=xt[c, :, 0:1024])
            for c in range(3):
                nc.gpsimd.dma_start(out=xq_sb[c][:, 0:512], in_=xq[c, :, 0:512])
            for c in range(3):
                nc.sync.dma_start(out=wq_sb[:, c, 2 * C : 3 * C], in_=wqkv[c, :, 2 * C : 3 * C])
            for t in range(1, 4):
                for c in range(3):
                    nc.gpsimd.dma_start(
                        out=xt_sb[c][:, t * 1024 : (t + 1) * 1024],
                        in_=xt[c, :, t * 1024 : (t + 1) * 1024],
                    )
            for c in range(3):
                nc.gpsimd.dma_start(out=xq_sb[c][:, 512:NQ], in_=xq[c, :, 512:NQ])
            for hp in range(3):
                nc.gpsimd.dma_start(out=wp_sb[:, hp, :], in_=wproj[hp, :, :])
            nc.gpsimd.dma_start(out=bp_sb, in_=bproj[:, :])

            # ---- QKV projection units (cast engine chosen per call site) ----
            def v_unit(k, eng):
                vps = wpool.tile([128, C], F32, tag="work", name=f"vps{k}")
                for c in range(3):
                    nc.tensor.matmul(
                        vps,
                        xt_sb[c][:, k * KT : (k + 1) * KT],
                        wq_sb[:, c, 2 * C : 3 * C],
                        start=(c == 0),
                        stop=(c == 2),
                    )
                dst = vp_sb[k]
                if eng == "v":
                    nc.vector.tensor_copy(
                        out=dst[:, :, 0:D], in_=vps.rearrange("p (h d) -> p h d", d=D)
                    )
                else:
                    nc.scalar.copy(
                        out=dst[:, :, 0:D], in_=vps.rearrange("p (h d) -> p h d", d=D)
                    )
                nc.gpsimd.memset(dst[:, :, D : D + 1], 1.0)

            def kq_unit(kind, j, t, eng):
                kq = wpool.tile([128, 512], F32, tag="work", name=f"kq{kind}{j}_{t}")
                coff = C + j * 128 if kind == "k" else j * 128
                src = xt_sb if kind == "k" else xq_sb
                dstt = kt_sb[j] if kind == "k" else qt_sb[j]
                for c in range(3):
                    nc.tensor.matmul(
                        kq,
                        wq_sb[:, c, coff : coff + 128],
                        src[c][:, t * 512 : (t + 1) * 512],
                        start=(c == 0),
                        stop=(c == 2),
                    )
                if eng == "v":
                    nc.vector.tensor_copy(out=dstt[:, t * 512 : (t + 1) * 512], in_=kq)
                else:
                    nc.scalar.copy(out=dstt[:, t * 512 : (t + 1) * 512], in_=kq)

            # upfront: only head pair 0's first K/Q tiles; ALL other QKV
            # work drains into the attention stream (PE is in-order).
            kq_unit("k", 0, 0, "v")
            kq_unit("q", 0, 0, "s")

            units = []
            ks = [("k", 0, t) for t in range(1, N // 512)]
            for k in range(NKT):
                units.append(("v", k, 0))
                if k % 2 == 1 and ks:
                    units.append(ks.pop(0))
            units.extend(ks)
            for j in (1, 2):
                units.append(("k", j, 0))
                units.append(("q", j, 0))
                for t in range(1, N // 512):
                    units.append(("k", j, t))
                for t in range(1, NQ // 512):
                    units.append(("q", j, t))
            for t in range(1, NQ // 512):
                units.append(("q", 0, t))
            units.reverse()  # pop() from the end

            def emit_unit(u, eng):
                if u[0] == "v":
                    v_unit(u[1], eng)
                else:
                    kq_unit(u[0], u[1], u[2], eng)

            # ---- deferred output projection (head-pair contraction K=128) ----
            def make_proj(qb):
                q0 = qb * QB

                def emit(co):
                    y = wpool.tile([128, QB], F32, tag="work", name=f"y{qb}_{co}")
                    for hp in range(3):
                        nc.tensor.matmul(
                            y,
                            wp_sb[:, hp, co * 128 : (co + 1) * 128],
                            at_sb[hp][:, q0 : q0 + QB],
                            start=(hp == 0),
                            stop=(hp == 2),
                        )
                    ysb = ypool.tile([128, QB], F32, tag="y", name=f"ysb{qb}_{co}")
                    # fused bias add during the PSUM->SBUF copy
                    nc.scalar.add(ysb, y, bp_sb[:, co : co + 1])
                    nc.sync.dma_start(out=out_t[co, :, q0 : q0 + QB], in_=ysb)

                return emit

            pending_proj = None
            tqueue = []  # deferred (transpose + at-copy) work items

            def drain_transpose():
                t_att, t_hp, t_q0, t_qs = tqueue.pop(0)
                tp = wpool.tile(
                    [128, 128], F16, tag="work", name=f"tp{t_hp}_{t_q0}_{t_qs}"
                )
                nc.tensor.transpose(tp, t_att, ident)
                dst = at_sb[t_hp][:, t_q0 + t_qs * QS : t_q0 + (t_qs + 1) * QS]
                nc.scalar.copy(out=dst, in_=tp)

            # ---- attention: (query-512-block, head-pair) x 32 key tiles ----
            for qb in range(NQB):
                q0 = qb * QB
                for hp in range(3):
                    hA, hB = 2 * hp, 2 * hp + 1
                    # one PSUM bank per head: [128 queries, 4 qsubtiles, 65]
                    oaccA = opool.tile([128, NQS, D + 1], F32, tag="oaccA", name=f"oA{qb}_{hp}")
                    oaccB = opool.tile([128, NQS, D + 1], F32, tag="oaccB", name=f"oB{qb}_{hp}")
                    pendings = []

                    def emit_av(pk, pp):
                        for oacc, h, off in ((oaccA, hA, 0), (oaccB, hB, QB)):
                            for qs in range(NQS):
                                nc.tensor.matmul(
                                    oacc[:, qs, :],
                                    pp[:, off + qs * QS : off + (qs + 1) * QS],
                                    vp_sb[pk][:, h, :],
                                    start=(pk == 0 and qs == 0),
                                    stop=(pk == NKT - 1 and qs == NQS - 1),
                                    skip_group_check=True,
                                )

                    for k in range(NKT):
                        exact = EXACT_TILE[k]
                        cast_eng = "v" if exact else "s"
                        s = spool.tile([128, 2 * QB], F32, tag="s")
                        nc.tensor.matmul(
                            s[:, 0:QB],
                            kt_sb[hp][0:D, k * KT : (k + 1) * KT],
                            qt_sb[hp][0:D, q0 : q0 + QB],
                            start=True,
                            stop=True,
                        )
                        nc.tensor.matmul(
                            s[:, QB : 2 * QB],
                            kt_sb[hp][D : 2 * D, k * KT : (k + 1) * KT],
                            qt_sb[hp][D : 2 * D, q0 : q0 + QB],
                            start=True,
                            stop=True,
                        )
                        p = ppool.tile([128, 2 * QB], F16, tag="p")
                        if exact:
                            nc.scalar.activation(
                                p, s, mybir.ActivationFunctionType.Exp, scale=SCALE
                            )
                        else:
                            nc.vector.tensor_scalar(
                                p.bitcast(I16),
                                s,
                                EXP_A,
                                EXP_B,
                                mybir.AluOpType.mult,
                                mybir.AluOpType.add,
                            )
                        if len(pendings) >= 2:
                            emit_av(*pendings.pop(0))
                        pendings.append((k, p))
                        if tqueue and k in (1, 3, 5, 7):
                            drain_transpose()
                        # drain deferred QKV work into the stream (PE slack)
                        if units and len(units) > 32:
                            emit_unit(units.pop(), cast_eng)
                            emit_unit(units.pop(), "v" if cast_eng == "s" else "s")
                        elif units and k % 2 == 1:
                            emit_unit(units.pop(), cast_eng)
                        if pending_proj is not None and hp == 0 and k in (8, 14, 20):
                            pending_proj({8: 0, 14: 1, 20: 2}[k])
                            if k == 20:
                                pending_proj = None
                    for pk, pp in pendings:
                        emit_av(pk, pp)
                    # normalize (1/Z per query partition) + PE transpose back
                    # to [dims, queries] for the projection
                    rzA = zpool.tile([128, NQS], F32, tag="rzA", name=f"rzA{qb}_{hp}")
                    rzB = zpool.tile([128, NQS], F32, tag="rzB", name=f"rzB{qb}_{hp}")
                    nc.vector.reciprocal(out=rzA, in_=oaccA[:, :, D])
                    nc.vector.reciprocal(out=rzB, in_=oaccB[:, :, D])
                    for qs in range(NQS):
                        att = apool.tile([128, 128], F16, tag="att", name=f"att{qb}_{hp}_{qs}")
                        nc.vector.tensor_scalar(
                            att[:, 0:D],
                            oaccA[:, qs, 0:D],
                            rzA[:, qs : qs + 1],
                            None,
                            mybir.AluOpType.mult,
                        )
                        nc.vector.tensor_scalar(
                            att[:, D : 2 * D],
                            oaccB[:, qs, 0:D],
                            rzB[:, qs : qs + 1],
                            None,
                            mybir.AluOpType.mult,
                        )
                        tqueue.append((att, hp, q0, qs))
                if qb < NQB - 1:
                    pending_proj = make_proj(qb)
            while tqueue:
                drain_transpose()
            final_proj = make_proj(NQB - 1)
            for co in range(3):
                final_proj(co)

    nc.compile()
    return nc


_NC_CACHE = {}


def _get_nc():
    if "nc" not in _NC_CACHE:
        _NC_CACHE["nc"] = _build()
    return _NC_CACHE["nc"]


def _prep_core_inputs(x, w_qkv, w_proj, b_proj):
    """Host-side sharding: returns in_maps for the 8 cores."""
    wqkv_p = np.ascontiguousarray(w_qkv.reshape(3, 128, 3 * C)).astype(F16_NP)
    wproj_p = np.ascontiguousarray(w_proj.reshape(3, 128, C)).astype(F16_NP)
    bproj_p = np.ascontiguousarray(b_proj.reshape(3, 128).T).astype(np.float32)
    in_maps = []
    for core in range(NCORES):
        b, qh = core // 2, core % 2
        xt_b = np.ascontiguousarray(x[b].T).astype(F16_NP)  # [C, N]
        xq_b = np.ascontiguousarray(x[b, qh * NQ : (qh + 1) * NQ].T).astype(F16_NP)
        in_maps.append(
            {
                "xt": xt_b.reshape(3, 128, N),
                "xq": xq_b.reshape(3, 128, NQ),
                "wqkv": wqkv_p,
                "wproj": wproj_p,
                "bproj": bproj_p,
            }
        )
    return in_maps


def run(inputs, **kw):
    """Run the kernel; returns (full_output, BassKernelResults)."""
    x = np.asarray(inputs["x"], dtype=np.float32)
    w_qkv = np.asarray(inputs["w_qkv"], dtype=np.float32)
    w_proj = np.asarray(inputs["w_proj"], dtype=np.float32)
    b_proj = np.asarray(inputs["b_proj"], dtype=np.float32)

    nc = _get_nc()
    in_maps = _prep_core_inputs(x, w_qkv, w_proj, b_proj)
    res = run_bass_kernel_spmd(nc, in_maps, core_ids=list(range(NCORES)), **kw)

    out = np.empty((B, N, C), dtype=np.float32)
    for core in range(NCORES):
        b, qh = core // 2, core % 2
        yt = res.results[core]["out_t"].reshape(C, NQ)  # [3*128, NQ]
        out[b, qh * NQ : (qh + 1) * NQ, :] = yt.T
    return out, res


def kernel(**inputs) -> np.ndarray:
    out, _ = run(inputs)
    return out
